# revision 19
# baseline (speedup 1.0000x reference)
"""Trainium2 Bass kernel for nn_BaselineMNISTClassifier (vq_codebook).

reference:
    x = samples - 0.5                        # [B, F]
    hv = einsum('bf,df->bd', x, bhv)         # [B, D]
    e = (hv > 0)                             # binary
    ham[b, c] = sum_d |e - centroids[c, d]|  # [B, C]
    return -ham

Only the SIGN of hv survives the binarize, so the encode matmul runs in
fp8 e4m3 (measured 0.91% bit-flip rate vs the f32 reference -> final
rel err ~8e-3, well under the 2e-2 gate). fp8 enables the PE's
DoubleRow perf mode: 256 contraction rows per matmul at 2 rows/cycle,
i.e. 4x the f32r rate the previous version used (256 cycles for a
256x128x512 matmul vs 512 cycles for 128x128x512).

Identity for the Hamming stage: with e' = (hv > 0) - 0.5 in {+-1/2} and
cmod = 1 - 2c in {-1, 0, +1}:  |e - c| = e' * cmod + 1/2, so
    ham[b, c] = sum_d e'[b, d] * cmod[c, d] + D/2
a second tiny matmul over the same d-tiles, also fp8 DoubleRow (exact:
all values are +-0.5/+-1/0). The binarize alternates DVE (is_gt-sub ->
e' = +-0.5) and ACT (Sign -> +-1) per d-tile; the matching 1.0 / 0.5
cmod scale is baked into the host-prepared centroid weights per d-tile
so both conventions contribute identically.

Sharding: D axis (10000) split across 8 cores, 1250 (padded 1280) per
core; every core sees the full batch, partial hammings sum on host.
F = 784 is zero-padded to 1024 = 4 chunks x (2 ktiles x 128 rows) so
every encode matmul is a uniform full-width DoubleRow op.

All quantization/transposition happens on host: x8 = fp8(64*(x-0.5)),
w8 = fp8(64*w) (the 64x scaling keeps values away from fp8 subnormals;
sign(hv) is scale-invariant), cmod in fp8 exactly.

Perf structure (per core):
  - warmup matmuls release the PE HAM clock gate while inputs stream
  - 2 b-groups of 4 blocks x 512; per (d-tile, block) 4 DoubleRow
    matmuls (q-chunks) accumulate in one PSUM bank; 7-bank rotation
  - consecutive matmuls over the 4 blocks share stationary weights,
    hiding LDWEIGHTS
  - hamming matmuls for d-pair P issue one d-tile late so the PE never
    waits on the binarize; all 4 accumulators of a b-group live in ONE
    PSUM bank at partition offsets 0/32/64/96 (tile_position)
  - last d-tile binarizes in halves; final hamming + epilogue drain
    per-block, outputs DMA out as their accumulation closes
"""

import sys

sys.path.insert(0, "/opt/trn_rl_repo")

import ml_dtypes
import numpy as np

import concourse.bacc as bacc
import concourse.bass as bass
import concourse.mybir as mybir
import concourse.tile as tile
from concourse.bass_utils import run_bass_kernel_spmd

B = 4096
F = 784
FP = 1024                    # F zero-padded: 4 chunks x (2 ktiles x 128)
NQ = 4                       # k-chunks of 256 (DoubleRow contraction)
D = 10000
C = 10
NCORES = 8
DREAL = D // NCORES          # 1250 real dims per core
DP = 1280                    # padded to 10 d-tiles of 128
ND = DP // 128               # 10
NPAIR = ND // 2              # 5 hamming d-pairs
NB = B // 512                # 8 b-blocks of 512
HC = 16                      # hamming stationary cols padded 10 -> 16
NWARM = 8                    # PE warmup matmuls

F32 = mybir.dt.float32
FP8 = mybir.dt.float8e4
OP = mybir.AluOpType
AF = mybir.ActivationFunctionType
PM = mybir.MatmulPerfMode

NP_FP8 = ml_dtypes.float8_e4m3
XSCALE = 64.0

# binarize engine per d-tile: 'v' = DVE is_gt-sub (e' = +-0.5, cmod +-1),
# 's' = ACT Sign (e2 = +-1, cmod +-0.5). Host bakes the matching cmod
# scale per d-tile (CM_SCALE).
BIN_ENG = ["v", "s", "v", "s", "v", "s", "v", "s", "s", "v"]
CM_SCALE = {"v": 1.0, "s": 0.5, "g": 1.0}

_NC_CACHE = {}


def _build_nc():
    if "nc" in _NC_CACHE:
        return _NC_CACHE["nc"]
    nc = bacc.Bacc("TRN2", debug=False, target_bir_lowering=False)
    x8 = nc.dram_tensor("x8", [FP, B], FP8, kind="ExternalInput")
    # stationary operands come pre-interleaved for DoubleRowSwInterleave:
    # per partition, position 2*(M-1-m)+t holds the ktile-t weight of
    # output column m (the HW dual-fp8 LDWEIGHTS layout, probe-verified)
    w8 = nc.dram_tensor("w8", [128, ND * NQ * 256], FP8,
                        kind="ExternalInput")
    cm8 = nc.dram_tensor("cm8", [128, ND * HC], FP8,
                         kind="ExternalInput")
    out = nc.dram_tensor("out", [C, B], F32, kind="ExternalOutput")

    with tile.TileContext(nc) as tc:
        with (
            tc.tile_pool(name="dum", bufs=2) as dumpool,
            tc.tile_pool(name="xp", bufs=2) as xpool,
            tc.tile_pool(name="wp", bufs=1) as wpool,
            tc.tile_pool(name="cp", bufs=1) as cpool,
            tc.tile_pool(name="ep", bufs=12) as epool,
            tc.tile_pool(name="op", bufs=4) as opool,
            tc.tile_pool(name="pse", bufs=7, space="PSUM") as psepool,
            tc.tile_pool(name="ps2", bufs=1, space="PSUM") as ps2pool,
        ):
            # --- input loads first: x bg0 is the critical path to the
            # first encode matmul, so it triggers before everything else;
            # w splits by d-tile (layout [p, di, q, m]) so early d-tiles
            # unblock while the rest streams
            xt0 = xpool.tile([128, NQ, 2, 2048], FP8, name="xt0", tag="xt")
            nc.sync.dma_start(
                xt0[:], x8[:, 0:2048].rearrange(
                    "(a t p) c -> p a t c", p=128, t=2))
            wt = wpool.tile([128, ND, NQ, 256], FP8)
            for di in range(ND):
                nc.gpsimd.dma_start(
                    wt[:, di, :, :],
                    w8[:, di * NQ * 256:(di + 1) * NQ * 256].rearrange(
                        "p (q m) -> p q m", q=NQ))
            ct = cpool.tile([128, ND, HC], FP8)
            nc.gpsimd.dma_start(
                ct[:], cm8.ap().rearrange("p (a m) -> p a m", a=ND))
            # xt1 (b-group 1) triggers later, from the DVE queue after the
            # first binarize round, so it does not steal HBM bandwidth from
            # the critical xt0 load
            xt1 = xpool.tile([128, NQ, 2, 2048], FP8, name="xt1", tag="xt")
            xts = [xt0, xt1]
            xt1_trig = [False]

            def trigger_xt1():
                # issued on the ACT queue after the di=1 Sign round, so it
                # fires only once the critical xt0 stream has finished
                nc.scalar.dma_start(
                    xt1[:], x8[:, 2048:4096].rearrange(
                        "(a t p) c -> p a t c", p=128, t=2))
                xt1_trig[0] = True

            # --- PE warmup: ramp the PE clock while inputs stream; the
            # dummies memset on DVE so the GpSimd queue stays free for
            # DMA triggers
            wdum = dumpool.tile([128, 256], FP8)
            nc.vector.memset(wdum[:], 1.0)
            xdum = dumpool.tile([128, 2, 512], FP8)
            nc.vector.memset(xdum[:], 1.0)
            psdum = psepool.tile([128, 512], F32, name="psdum", tag="pse")
            for i in range(NWARM):
                nc.tensor.matmul(psdum[:], wdum[:], xdum[:],
                                 start=(i == 0), stop=(i == NWARM - 1),
                                 perf_mode=PM.DoubleRowSwInterleave)

            # --- main compute: two b-groups of 4 blocks of 512.
            for bg in range(2):
                ps2 = ps2pool.tile([128, 512], F32, name=f"ps2_{bg}",
                                   tag="ps2")
                psum2 = [ps2[32 * j:32 * j + HC, :] for j in range(4)]
                psum2o = [ps2[32 * j:32 * j + C, :] for j in range(4)]
                pending = []
                for di in range(ND):
                    pses = [
                        psepool.tile([128, 512], F32,
                                     name=f"pse_{di % 2}_{j}", tag="pse")
                        for j in range(4)
                    ]
                    for q in range(NQ):
                        wq = wt[:, di, q, :]
                        for j in range(4):
                            nc.tensor.matmul(
                                pses[j][:], wq,
                                xts[bg][:, q, :, j * 512:(j + 1) * 512],
                                start=(q == 0), stop=(q == NQ - 1),
                                perf_mode=PM.DoubleRowSwInterleave)
                    # hamming for the previous d-tile: issued here so the
                    # PE reaches it well after its binarize completes
                    for pdi, pets in pending:
                        for j in range(4):
                            nc.tensor.matmul(
                                psum2[j], ct[:, pdi, :], pets[j][:],
                                start=(pdi == 0), stop=False,
                                tile_position=(0, 32 * j))
                    pending = []
                    # binarize this d-tile (engine per BIN_ENG); the last
                    # d-tile goes in halves so the final hamming overlaps
                    eng = BIN_ENG[di]
                    ets = [epool.tile([128, 512], FP8,
                                      name=f"et_{di % 3}_{j}", tag="et")
                           for j in range(4)]
                    for j in range(4):
                        sls = ([slice(0, 256), slice(256, 512)]
                               if di == ND - 1 else [slice(0, 512)])
                        for sl in sls:
                            if eng == "v":
                                nc.vector.tensor_scalar(
                                    ets[j][:, sl], pses[j][:, sl], 0.0, 0.5,
                                    op0=OP.is_gt, op1=OP.subtract)
                            else:
                                nc.scalar.activation(
                                    ets[j][:, sl], pses[j][:, sl], AF.Sign)
                    if bg == 0 and di == 1:
                        trigger_xt1()
                    pending.append((di, ets))
                # final d-tile: hamming in halves, epilogue + output DMA
                # per block as each accumulation closes
                pdi, pets = pending[0]
                for j in range(4):
                    for h in range(2):
                        sl = slice(h * 256, (h + 1) * 256)
                        nc.tensor.matmul(
                            psum2[j][:, sl], ct[:, pdi, :],
                            pets[j][:, sl],
                            start=False, stop=True,
                            tile_position=(0, 32 * j))
                    # out = -(psum2 + DREAL/2); alternate engines so the
                    # four epilogues drain in parallel
                    ot = opool.tile([C, 512], F32, name=f"ot_{j}", tag="ot")
                    if j % 2 == 0:
                        nc.vector.tensor_scalar(ot[:], psum2o[j],
                                                float(DREAL) / 2.0, -1.0,
                                                op0=OP.add, op1=OP.mult)
                    else:
                        nc.scalar.activation(ot[:], psum2o[j], AF.Copy,
                                             bias=-float(DREAL) / 2.0,
                                             scale=-1.0)
                    nc.gpsimd.dma_start(
                        out[:, (bg * 4 + j) * 512:(bg * 4 + j + 1) * 512],
                        ot[:])
    nc.compile()
    _NC_CACHE["nc"] = nc
    return nc


def _prep_in_maps(samples, bhv_matrix, centroids):
    samples = np.ascontiguousarray(samples, dtype=np.float32)
    bhv_matrix = np.ascontiguousarray(bhv_matrix, dtype=np.float32)
    centroids = np.ascontiguousarray(centroids, dtype=np.float32)

    # x8 [FP, B]: row f = q*256 + t*128 + r; fp8(64*(x - 0.5)), 0-padded
    xz = np.zeros((FP, B), dtype=np.float32)
    xz[:F, :] = (samples.T - 0.5) * XSCALE
    x_all = xz.astype(NP_FP8)

    cm_scale = np.zeros((DP, 1), dtype=np.float32)
    for di in range(ND):
        cm_scale[di * 128:(di + 1) * 128] = CM_SCALE[BIN_ENG[di]]

    in_maps = []
    for k in range(NCORES):
        lo, hi = k * DREAL, (k + 1) * DREAL
        wz = np.zeros((FP, DP), dtype=np.float32)
        wz[:F, :DREAL] = bhv_matrix[lo:hi, :].T * XSCALE
        # SwInterleave stationary layout: [r, q, di, m, t] with m reversed,
        # flattened so position 2*(127-m)+t holds (f=q*256+t*128+r, d=di*128+m)
        wv = wz.reshape(NQ, 2, 128, ND, 128).transpose(2, 3, 0, 4, 1)
        wv = wv[:, :, :, ::-1, :]
        w8 = np.ascontiguousarray(wv).reshape(128, ND * NQ * 256)
        w8 = w8.astype(NP_FP8)
        cz = np.zeros((DP, HC), dtype=np.float32)
        cz[:DREAL, :C] = 1.0 - 2.0 * centroids[:, lo:hi].T
        cz *= cm_scale
        cv = cz.reshape(ND, 128, HC).transpose(1, 0, 2)
        cm8 = np.ascontiguousarray(cv).reshape(128, ND * HC)
        cm8 = cm8.astype(NP_FP8)
        in_maps.append({"x8": x_all, "w8": w8, "cm8": cm8})
    return in_maps


def _run(samples, bhv_matrix, centroids, **spmd_kwargs):
    nc = _build_nc()
    in_maps = _prep_in_maps(samples, bhv_matrix, centroids)
    res = run_bass_kernel_spmd(nc, in_maps, core_ids=list(range(NCORES)),
                               **spmd_kwargs)
    acc = np.zeros((C, B), dtype=np.float32)
    for r in res.results:
        acc += r["out"]
    return np.ascontiguousarray(acc.T), res


def kernel(samples, bhv_matrix, centroids):
    out, _ = _run(samples, bhv_matrix, centroids)
    return out


# revision 20
# speedup vs baseline: 1.0334x; 1.0334x over previous
"""Trainium2 Bass kernel for nn_BaselineMNISTClassifier (vq_codebook).

reference:
    x = samples - 0.5                        # [B, F]
    hv = einsum('bf,df->bd', x, bhv)         # [B, D]
    e = (hv > 0)                             # binary
    ham[b, c] = sum_d |e - centroids[c, d]|  # [B, C]
    return -ham

Only the SIGN of hv survives the binarize, so the encode matmul runs in
fp8 e4m3 (measured 0.91% bit-flip rate vs the f32 reference -> final
rel err ~8e-3, well under the 2e-2 gate). fp8 enables the PE's
DoubleRow perf mode: 256 contraction rows per matmul at 2 rows/cycle,
i.e. 4x the f32r rate the previous version used (256 cycles for a
256x128x512 matmul vs 512 cycles for 128x128x512).

Identity for the Hamming stage: with e' = (hv > 0) - 0.5 in {+-1/2} and
cmod = 1 - 2c in {-1, 0, +1}:  |e - c| = e' * cmod + 1/2, so
    ham[b, c] = sum_d e'[b, d] * cmod[c, d] + D/2
a second tiny matmul over the same d-tiles, also fp8 DoubleRow (exact:
all values are +-0.5/+-1/0). The binarize alternates DVE (is_gt-sub ->
e' = +-0.5) and ACT (Sign -> +-1) per d-tile; the matching 1.0 / 0.5
cmod scale is baked into the host-prepared centroid weights per d-tile
so both conventions contribute identically.

Sharding: D axis (10000) split across 8 cores, 1250 (padded 1280) per
core; every core sees the full batch, partial hammings sum on host.
F = 784 is zero-padded to 1024 = 4 chunks x (2 ktiles x 128 rows) so
every encode matmul is a uniform full-width DoubleRow op.

All quantization/transposition happens on host: x8 = fp8(64*(x-0.5)),
w8 = fp8(64*w) (the 64x scaling keeps values away from fp8 subnormals;
sign(hv) is scale-invariant), cmod in fp8 exactly.

Perf structure (per core):
  - warmup matmuls release the PE HAM clock gate while inputs stream
  - 2 b-groups of 4 blocks x 512; per (d-tile, block) 4 DoubleRow
    matmuls (q-chunks) accumulate in one PSUM bank; 7-bank rotation
  - consecutive matmuls over the 4 blocks share stationary weights,
    hiding LDWEIGHTS
  - hamming matmuls for d-pair P issue one d-tile late so the PE never
    waits on the binarize; all 4 accumulators of a b-group live in ONE
    PSUM bank at partition offsets 0/32/64/96 (tile_position)
  - last d-tile binarizes in halves; final hamming + epilogue drain
    per-block, outputs DMA out as their accumulation closes
"""

import sys

sys.path.insert(0, "/opt/trn_rl_repo")

import ml_dtypes
import numpy as np

import concourse.bacc as bacc
import concourse.bass as bass
import concourse.mybir as mybir
import concourse.tile as tile
from concourse.bass_utils import run_bass_kernel_spmd

B = 4096
F = 784
FP = 1024                    # F zero-padded: 4 chunks x (2 ktiles x 128)
NQ = 4                       # k-chunks of 256 (DoubleRow contraction)
D = 10000
C = 10
NCORES = 8
DREAL = D // NCORES          # 1250 real dims per core
DP = 1280                    # padded to 10 d-tiles of 128
ND = DP // 128               # 10
NPAIR = ND // 2              # 5 hamming d-pairs
NB = B // 512                # 8 b-blocks of 512
HC = 16                      # hamming stationary cols padded 10 -> 16
NWARM = 8                    # PE warmup matmuls

F32 = mybir.dt.float32
FP8 = mybir.dt.float8e4
OP = mybir.AluOpType
AF = mybir.ActivationFunctionType
PM = mybir.MatmulPerfMode

NP_FP8 = ml_dtypes.float8_e4m3
XSCALE = 64.0

# binarize engine per d-tile: 'v' = DVE is_gt-sub (e' = +-0.5, cmod +-1),
# 's' = ACT Sign (e2 = +-1, cmod +-0.5). Host bakes the matching cmod
# scale per d-tile (CM_SCALE).
BIN_ENG = ["v", "s", "v", "s", "v", "s", "v", "s", "s", "v"]
CM_SCALE = {"v": 1.0, "s": 0.5, "g": 1.0}

_NC_CACHE = {}


def _build_nc():
    if "nc" in _NC_CACHE:
        return _NC_CACHE["nc"]
    nc = bacc.Bacc("TRN2", debug=False, target_bir_lowering=False)
    # x8 rows 0-127 = b-group 0 partitions, 128-255 = b-group 1; each
    # row is one SBUF partition's full 16KB (contiguous -> one DMA
    # descriptor per partition instead of 8)
    x8 = nc.dram_tensor("x8", [2 * 128, NQ * 2 * 2048], FP8,
                        kind="ExternalInput")
    # stationary operands come pre-interleaved for DoubleRowSwInterleave:
    # per partition, position 2*(M-1-m)+t holds the ktile-t weight of
    # output column m (the HW dual-fp8 LDWEIGHTS layout, probe-verified)
    w8 = nc.dram_tensor("w8", [128, ND * NQ * 256], FP8,
                        kind="ExternalInput")
    cm8 = nc.dram_tensor("cm8", [128, ND * HC], FP8,
                         kind="ExternalInput")
    out = nc.dram_tensor("out", [C, B], F32, kind="ExternalOutput")

    with tile.TileContext(nc) as tc:
        with (
            tc.tile_pool(name="dum", bufs=2) as dumpool,
            tc.tile_pool(name="xp", bufs=2) as xpool,
            tc.tile_pool(name="wp", bufs=1) as wpool,
            tc.tile_pool(name="cp", bufs=1) as cpool,
            tc.tile_pool(name="ep", bufs=12) as epool,
            tc.tile_pool(name="op", bufs=4) as opool,
            tc.tile_pool(name="pse", bufs=7, space="PSUM") as psepool,
            tc.tile_pool(name="ps2", bufs=1, space="PSUM") as ps2pool,
        ):
            # --- input loads first: x bg0 is the critical path to the
            # first encode matmul, so it triggers before everything else;
            # w splits by d-tile (layout [p, di, q, m]) so early d-tiles
            # unblock while the rest streams
            xt0 = xpool.tile([128, NQ, 2, 2048], FP8, name="xt0", tag="xt")
            nc.sync.dma_start(
                xt0[:], x8[0:128, :].rearrange(
                    "p (q t c) -> p q t c", q=NQ, t=2))
            wt = wpool.tile([128, ND, NQ, 256], FP8)
            for lo, hi in ((0, 2), (2, 6), (6, ND)):
                nc.gpsimd.dma_start(
                    wt[:, lo:hi, :, :],
                    w8[:, lo * NQ * 256:hi * NQ * 256].rearrange(
                        "p (a q m) -> p a q m", a=hi - lo, q=NQ))
            ct = cpool.tile([128, ND, HC], FP8)
            nc.gpsimd.dma_start(
                ct[:], cm8.ap().rearrange("p (a m) -> p a m", a=ND))
            # xt1 (b-group 1) triggers later, from the DVE queue after the
            # first binarize round, so it does not steal HBM bandwidth from
            # the critical xt0 load
            xt1 = xpool.tile([128, NQ, 2, 2048], FP8, name="xt1", tag="xt")
            xts = [xt0, xt1]
            xt1_trig = [False]

            def trigger_xt1():
                # issued on the ACT queue after the di=1 Sign round, so it
                # fires only once the critical xt0 stream has finished
                nc.scalar.dma_start(
                    xt1[:], x8[128:256, :].rearrange(
                        "p (q t c) -> p q t c", q=NQ, t=2))
                xt1_trig[0] = True

            # --- PE warmup: ramp the PE clock while inputs stream; the
            # dummies memset on DVE so the GpSimd queue stays free for
            # DMA triggers
            wdum = dumpool.tile([128, 256], FP8)
            nc.vector.memset(wdum[:], 1.0)
            xdum = dumpool.tile([128, 2, 512], FP8)
            nc.vector.memset(xdum[:], 1.0)
            psdum = psepool.tile([128, 512], F32, name="psdum", tag="pse")
            for i in range(NWARM):
                nc.tensor.matmul(psdum[:], wdum[:], xdum[:],
                                 start=(i == 0), stop=(i == NWARM - 1),
                                 perf_mode=PM.DoubleRowSwInterleave)

            # --- main compute: two b-groups of 4 blocks of 512.
            for bg in range(2):
                ps2 = ps2pool.tile([128, 512], F32, name=f"ps2_{bg}",
                                   tag="ps2")
                psum2 = [ps2[32 * j:32 * j + HC, :] for j in range(4)]
                psum2o = [ps2[32 * j:32 * j + C, :] for j in range(4)]
                pending = []
                for di in range(ND):
                    pses = [
                        psepool.tile([128, 512], F32,
                                     name=f"pse_{di % 2}_{j}", tag="pse")
                        for j in range(4)
                    ]
                    for q in range(NQ):
                        wq = wt[:, di, q, :]
                        for j in range(4):
                            nc.tensor.matmul(
                                pses[j][:], wq,
                                xts[bg][:, q, :, j * 512:(j + 1) * 512],
                                start=(q == 0), stop=(q == NQ - 1),
                                perf_mode=PM.DoubleRowSwInterleave)
                    # hamming for the previous d-tile: issued here so the
                    # PE reaches it well after its binarize completes
                    for pdi, pets in pending:
                        for j in range(4):
                            nc.tensor.matmul(
                                psum2[j], ct[:, pdi, :], pets[j][:],
                                start=(pdi == 0), stop=False,
                                tile_position=(0, 32 * j))
                    pending = []
                    # binarize this d-tile (engine per BIN_ENG); the last
                    # d-tile goes in halves so the final hamming overlaps
                    eng = BIN_ENG[di]
                    ets = [epool.tile([128, 512], FP8,
                                      name=f"et_{di % 3}_{j}", tag="et")
                           for j in range(4)]
                    for j in range(4):
                        sls = ([slice(0, 256), slice(256, 512)]
                               if di == ND - 1 else [slice(0, 512)])
                        for sl in sls:
                            if eng == "v":
                                nc.vector.tensor_scalar(
                                    ets[j][:, sl], pses[j][:, sl], 0.0, 0.5,
                                    op0=OP.is_gt, op1=OP.subtract)
                            else:
                                nc.scalar.activation(
                                    ets[j][:, sl], pses[j][:, sl], AF.Sign)
                    if bg == 0 and di == 1:
                        trigger_xt1()
                    pending.append((di, ets))
                # final d-tile: hamming in halves, epilogue + output DMA
                # per block as each accumulation closes
                pdi, pets = pending[0]
                for j in range(4):
                    for h in range(2):
                        sl = slice(h * 256, (h + 1) * 256)
                        nc.tensor.matmul(
                            psum2[j][:, sl], ct[:, pdi, :],
                            pets[j][:, sl],
                            start=False, stop=True,
                            tile_position=(0, 32 * j))
                    # out = -(psum2 + DREAL/2); alternate engines so the
                    # four epilogues drain in parallel
                    ot = opool.tile([C, 512], F32, name=f"ot_{j}", tag="ot")
                    if j % 2 == 0:
                        nc.vector.tensor_scalar(ot[:], psum2o[j],
                                                float(DREAL) / 2.0, -1.0,
                                                op0=OP.add, op1=OP.mult)
                    else:
                        nc.scalar.activation(ot[:], psum2o[j], AF.Copy,
                                             bias=-float(DREAL) / 2.0,
                                             scale=-1.0)
                    nc.gpsimd.dma_start(
                        out[:, (bg * 4 + j) * 512:(bg * 4 + j + 1) * 512],
                        ot[:])
    nc.compile()
    _NC_CACHE["nc"] = nc
    return nc


def _prep_in_maps(samples, bhv_matrix, centroids):
    samples = np.ascontiguousarray(samples, dtype=np.float32)
    bhv_matrix = np.ascontiguousarray(bhv_matrix, dtype=np.float32)
    centroids = np.ascontiguousarray(centroids, dtype=np.float32)

    # x8 [2*128, 16KB]: [bg*128 + r, (q*2+t)*2048 + c] =
    # fp8(64*(x - 0.5))[f = q*256+t*128+r, b = bg*2048+c], 0-padded
    xz = np.zeros((FP, B), dtype=np.float32)
    xz[:F, :] = (samples.T - 0.5) * XSCALE
    xv = xz.reshape(NQ, 2, 128, 2, 2048).transpose(3, 2, 0, 1, 4)
    x_all = np.ascontiguousarray(xv).reshape(2 * 128, NQ * 2 * 2048)
    x_all = x_all.astype(NP_FP8)

    cm_scale = np.zeros((DP, 1), dtype=np.float32)
    for di in range(ND):
        cm_scale[di * 128:(di + 1) * 128] = CM_SCALE[BIN_ENG[di]]

    in_maps = []
    for k in range(NCORES):
        lo, hi = k * DREAL, (k + 1) * DREAL
        wz = np.zeros((FP, DP), dtype=np.float32)
        wz[:F, :DREAL] = bhv_matrix[lo:hi, :].T * XSCALE
        # SwInterleave stationary layout: [r, q, di, m, t] with m reversed,
        # flattened so position 2*(127-m)+t holds (f=q*256+t*128+r, d=di*128+m)
        wv = wz.reshape(NQ, 2, 128, ND, 128).transpose(2, 3, 0, 4, 1)
        wv = wv[:, :, :, ::-1, :]
        w8 = np.ascontiguousarray(wv).reshape(128, ND * NQ * 256)
        w8 = w8.astype(NP_FP8)
        cz = np.zeros((DP, HC), dtype=np.float32)
        cz[:DREAL, :C] = 1.0 - 2.0 * centroids[:, lo:hi].T
        cz *= cm_scale
        cv = cz.reshape(ND, 128, HC).transpose(1, 0, 2)
        cm8 = np.ascontiguousarray(cv).reshape(128, ND * HC)
        cm8 = cm8.astype(NP_FP8)
        in_maps.append({"x8": x_all, "w8": w8, "cm8": cm8})
    return in_maps


def _run(samples, bhv_matrix, centroids, **spmd_kwargs):
    nc = _build_nc()
    in_maps = _prep_in_maps(samples, bhv_matrix, centroids)
    res = run_bass_kernel_spmd(nc, in_maps, core_ids=list(range(NCORES)),
                               **spmd_kwargs)
    acc = np.zeros((C, B), dtype=np.float32)
    for r in res.results:
        acc += r["out"]
    return np.ascontiguousarray(acc.T), res


def kernel(samples, bhv_matrix, centroids):
    out, _ = _run(samples, bhv_matrix, centroids)
    return out


# revision 23
# speedup vs baseline: 1.1257x; 1.0893x over previous
"""Trainium2 Bass kernel for nn_BaselineMNISTClassifier (vq_codebook).

reference:
    x = samples - 0.5                        # [B, F]
    hv = einsum('bf,df->bd', x, bhv)         # [B, D]
    e = (hv > 0)                             # binary
    ham[b, c] = sum_d |e - centroids[c, d]|  # [B, C]
    return -ham

Only the SIGN of hv survives the binarize, so the encode matmul runs in
fp8 e4m3 (measured 0.91% bit-flip rate vs the f32 reference -> final
rel err ~8e-3, well under the 2e-2 gate). fp8 enables the PE's
DoubleRow perf mode: 256 contraction rows per matmul at 2 rows/cycle,
i.e. 4x the f32r rate the previous version used (256 cycles for a
256x128x512 matmul vs 512 cycles for 128x128x512).

Identity for the Hamming stage: with e' = (hv > 0) - 0.5 in {+-1/2} and
cmod = 1 - 2c in {-1, 0, +1}:  |e - c| = e' * cmod + 1/2, so
    ham[b, c] = sum_d e'[b, d] * cmod[c, d] + D/2
a second tiny matmul over the same d-tiles, also fp8 DoubleRow (exact:
all values are +-0.5/+-1/0). The binarize alternates DVE (is_gt-sub ->
e' = +-0.5) and ACT (Sign -> +-1) per d-tile; the matching 1.0 / 0.5
cmod scale is baked into the host-prepared centroid weights per d-tile
so both conventions contribute identically.

Sharding: D axis (10000) split across 8 cores, 1250 (padded 1280) per
core; every core sees the full batch, partial hammings sum on host.
F = 784 is zero-padded to 1024 = 4 chunks x (2 ktiles x 128 rows) so
every encode matmul is a uniform full-width DoubleRow op.

All quantization/transposition happens on host: x8 = fp8(64*(x-0.5)),
w8 = fp8(64*w) (the 64x scaling keeps values away from fp8 subnormals;
sign(hv) is scale-invariant), cmod in fp8 exactly.

Perf structure (per core):
  - warmup matmuls release the PE HAM clock gate while inputs stream
  - 2 b-groups of 4 blocks x 512; per (d-tile, block) 4 DoubleRow
    matmuls (q-chunks) accumulate in one PSUM bank; 7-bank rotation
  - consecutive matmuls over the 4 blocks share stationary weights,
    hiding LDWEIGHTS
  - hamming matmuls for d-pair P issue one d-tile late so the PE never
    waits on the binarize; all 4 accumulators of a b-group live in ONE
    PSUM bank at partition offsets 0/32/64/96 (tile_position)
  - last d-tile binarizes in halves; final hamming + epilogue drain
    per-block, outputs DMA out as their accumulation closes
"""

import sys

sys.path.insert(0, "/opt/trn_rl_repo")

import ml_dtypes
import numpy as np

import concourse.bacc as bacc
import concourse.bass as bass
import concourse.mybir as mybir
import concourse.tile as tile
from concourse.bass_utils import run_bass_kernel_spmd

B = 4096
F = 784
FP = 1024                    # F zero-padded: 4 chunks x (2 ktiles x 128)
NQ = 4                       # k-chunks of 256 (DoubleRow contraction)
D = 10000
C = 10
NCORES = 8
DREAL = D // NCORES          # 1250 real dims per core
DP = 1280                    # padded to 10 d-tiles of 128
ND = DP // 128               # 10
NPAIR = ND // 2              # 5 hamming d-pairs
NB = B // 512                # 8 b-blocks of 512
HC = 16                      # hamming stationary cols padded 10 -> 16
NWARM = 16                   # PE warmup matmuls

F32 = mybir.dt.float32
FP8 = mybir.dt.float8e4
OP = mybir.AluOpType
AF = mybir.ActivationFunctionType
PM = mybir.MatmulPerfMode

NP_FP8 = ml_dtypes.float8_e4m3
XSCALE = 64.0

# binarize engine per b-block: even blocks on DVE (is_gt-sub, e' = +-0.5,
# cmod +-1), odd blocks on ACT (Sign, e2 = +-1, cmod +-0.5). The host
# provides both cmod scalings per d-tile so each hamming matmul picks the
# variant matching its block's binarize convention.

_NC_CACHE = {}


def _build_nc():
    if "nc" in _NC_CACHE:
        return _NC_CACHE["nc"]
    nc = bacc.Bacc("TRN2", debug=False, target_bir_lowering=False)
    # x8 rows 0-127 = b-group 0 partitions, 128-255 = b-group 1; each
    # row is one SBUF partition's full 16KB (contiguous -> one DMA
    # descriptor per partition instead of 8)
    x8 = nc.dram_tensor("x8", [2 * 128, NQ * 2 * 2048], FP8,
                        kind="ExternalInput")
    # stationary operands come pre-interleaved for DoubleRowSwInterleave:
    # per partition, position 2*(M-1-m)+t holds the ktile-t weight of
    # output column m (the HW dual-fp8 LDWEIGHTS layout, probe-verified)
    w8 = nc.dram_tensor("w8", [128, ND * NQ * 256], FP8,
                        kind="ExternalInput")
    cm8 = nc.dram_tensor("cm8", [128, ND * 2 * HC], FP8,
                         kind="ExternalInput")
    out = nc.dram_tensor("out", [C, B], F32, kind="ExternalOutput")

    with tile.TileContext(nc) as tc:
        with (
            tc.tile_pool(name="dum", bufs=2) as dumpool,
            tc.tile_pool(name="xp", bufs=2) as xpool,
            tc.tile_pool(name="wp", bufs=1) as wpool,
            tc.tile_pool(name="cp", bufs=1) as cpool,
            tc.tile_pool(name="ep", bufs=12) as epool,
            tc.tile_pool(name="op", bufs=4) as opool,
            tc.tile_pool(name="pse", bufs=7, space="PSUM") as psepool,
            tc.tile_pool(name="ps2", bufs=1, space="PSUM") as ps2pool,
        ):
            # --- input loads first: x bg0 is the critical path to the
            # first encode matmul, so it triggers before everything else;
            # w splits by d-tile (layout [p, di, q, m]) so early d-tiles
            # unblock while the rest streams
            xt0 = xpool.tile([128, NQ, 2, 2048], FP8, name="xt0", tag="xt")
            nc.sync.dma_start(
                xt0[:], x8[0:128, :].rearrange(
                    "p (q t c) -> p q t c", q=NQ, t=2))
            wt = wpool.tile([128, ND, NQ, 256], FP8)
            for lo, hi in ((0, 2), (2, 6), (6, ND)):
                nc.gpsimd.dma_start(
                    wt[:, lo:hi, :, :],
                    w8[:, lo * NQ * 256:hi * NQ * 256].rearrange(
                        "p (a q m) -> p a q m", a=hi - lo, q=NQ))
            ct = cpool.tile([128, ND, 2, HC], FP8)
            nc.gpsimd.dma_start(
                ct[:], cm8.ap().rearrange("p (a v m) -> p a v m",
                                          a=ND, v=2))
            # xt1 (b-group 1) triggers later, from the DVE queue after the
            # first binarize round, so it does not steal HBM bandwidth from
            # the critical xt0 load
            xt1 = xpool.tile([128, NQ, 2, 2048], FP8, name="xt1", tag="xt")
            xts = [xt0, xt1]
            xt1_trig = [False]

            def trigger_xt1():
                # issued on the ACT queue after the di=1 Sign round, so it
                # fires only once the critical xt0 stream has finished
                nc.scalar.dma_start(
                    xt1[:], x8[128:256, :].rearrange(
                        "p (q t c) -> p q t c", q=NQ, t=2))
                xt1_trig[0] = True

            # --- PE warmup: ramp the PE clock while inputs stream; the
            # dummies memset on DVE so the GpSimd queue stays free for
            # DMA triggers
            wdum = dumpool.tile([128, 256], FP8)
            nc.vector.memset(wdum[:], 1.0)
            xdum = dumpool.tile([128, 2, 512], FP8)
            nc.vector.memset(xdum[:], 1.0)
            psdum = psepool.tile([128, 512], F32, name="psdum", tag="pse")
            for i in range(NWARM):
                nc.tensor.matmul(psdum[:], wdum[:], xdum[:],
                                 start=(i == 0), stop=(i == NWARM - 1),
                                 perf_mode=PM.DoubleRowSwInterleave)

            # --- main compute: two b-groups of 4 blocks of 512.
            for bg in range(2):
                ps2 = ps2pool.tile([128, 512], F32, name=f"ps2_{bg}",
                                   tag="ps2")
                psum2 = [ps2[32 * j:32 * j + HC, :] for j in range(4)]
                psum2o = [ps2[32 * j:32 * j + C, :] for j in range(4)]
                pending = []
                for di in range(ND):
                    pses = [
                        psepool.tile([128, 512], F32,
                                     name=f"pse_{di % 2}_{j}", tag="pse")
                        for j in range(4)
                    ]
                    for q in range(NQ):
                        wq = wt[:, di, q, :]
                        for j in range(4):
                            nc.tensor.matmul(
                                pses[j][:], wq,
                                xts[bg][:, q, :, j * 512:(j + 1) * 512],
                                start=(q == 0), stop=(q == NQ - 1),
                                perf_mode=PM.DoubleRowSwInterleave)
                    # hamming for the previous d-tile: issued here so the
                    # PE reaches it well after its binarize completes
                    for pdi, pets in pending:
                        for j in range(4):
                            nc.tensor.matmul(
                                psum2[j], ct[:, pdi, j % 2, :], pets[j][:],
                                start=(pdi == 0), stop=False,
                                tile_position=(0, 32 * j))
                    pending = []
                    # binarize this d-tile (engine per BIN_ENG); the last
                    # d-tile goes in halves so the final hamming overlaps
                    ets = [epool.tile([128, 512], FP8,
                                      name=f"et_{di % 3}_{j}", tag="et")
                           for j in range(4)]
                    for j in range(4):
                        sls = ([slice(0, 256), slice(256, 512)]
                               if di == ND - 1 else [slice(0, 512)])
                        for sl in sls:
                            if j % 2 == 0:
                                nc.vector.tensor_scalar(
                                    ets[j][:, sl], pses[j][:, sl], 0.0, 0.5,
                                    op0=OP.is_gt, op1=OP.subtract)
                            else:
                                nc.scalar.activation(
                                    ets[j][:, sl], pses[j][:, sl], AF.Sign)
                    if bg == 0 and di == 1:
                        # tiny write into xt1 reading this d-tile's psum
                        # gives the xt1 DMA a write-after-write dependency,
                        # so the scheduler cannot hoist it into the middle
                        # of the critical xt0 stream
                        nc.vector.tensor_scalar_mul(
                            xt1[0:1, 0, 0, 0:1], pses[0][0:1, 0:1], 0.0)
                        trigger_xt1()
                    pending.append((di, ets))
                # final d-tile: hamming in halves, epilogue + output DMA
                # per block as each accumulation closes
                pdi, pets = pending[0]
                for j in range(4):
                    for h in range(2):
                        sl = slice(h * 256, (h + 1) * 256)
                        nc.tensor.matmul(
                            psum2[j][:, sl], ct[:, pdi, j % 2, :],
                            pets[j][:, sl],
                            start=False, stop=True,
                            tile_position=(0, 32 * j))
                    # out = -(psum2 + DREAL/2); alternate engines so the
                    # four epilogues drain in parallel
                    ot = opool.tile([C, 512], F32, name=f"ot_{j}", tag="ot")
                    if j % 2 == 0:
                        nc.vector.tensor_scalar(ot[:], psum2o[j],
                                                float(DREAL) / 2.0, -1.0,
                                                op0=OP.add, op1=OP.mult)
                    else:
                        nc.scalar.activation(ot[:], psum2o[j], AF.Copy,
                                             bias=-float(DREAL) / 2.0,
                                             scale=-1.0)
                    nc.gpsimd.dma_start(
                        out[:, (bg * 4 + j) * 512:(bg * 4 + j + 1) * 512],
                        ot[:])
    nc.compile()
    _NC_CACHE["nc"] = nc
    return nc


def _prep_in_maps(samples, bhv_matrix, centroids):
    samples = np.ascontiguousarray(samples, dtype=np.float32)
    bhv_matrix = np.ascontiguousarray(bhv_matrix, dtype=np.float32)
    centroids = np.ascontiguousarray(centroids, dtype=np.float32)

    # x8 [2*128, 16KB]: [bg*128 + r, (q*2+t)*2048 + c] =
    # fp8(64*(x - 0.5))[f = q*256+t*128+r, b = bg*2048+c], 0-padded
    xz = np.zeros((FP, B), dtype=np.float32)
    xz[:F, :] = (samples.T - 0.5) * XSCALE
    xv = xz.reshape(NQ, 2, 128, 2, 2048).transpose(3, 2, 0, 1, 4)
    x_all = np.ascontiguousarray(xv).reshape(2 * 128, NQ * 2 * 2048)
    x_all = x_all.astype(NP_FP8)

    in_maps = []
    for k in range(NCORES):
        lo, hi = k * DREAL, (k + 1) * DREAL
        wz = np.zeros((FP, DP), dtype=np.float32)
        wz[:F, :DREAL] = bhv_matrix[lo:hi, :].T * XSCALE
        # SwInterleave stationary layout: [r, q, di, m, t] with m reversed,
        # flattened so position 2*(127-m)+t holds (f=q*256+t*128+r, d=di*128+m)
        wv = wz.reshape(NQ, 2, 128, ND, 128).transpose(2, 3, 0, 4, 1)
        wv = wv[:, :, :, ::-1, :]
        w8 = np.ascontiguousarray(wv).reshape(128, ND * NQ * 256)
        w8 = w8.astype(NP_FP8)
        cz = np.zeros((DP, 2, HC), dtype=np.float32)
        cz[:DREAL, 0, :C] = 1.0 - 2.0 * centroids[:, lo:hi].T
        cz[:, 1, :] = 0.5 * cz[:, 0, :]
        cv = cz.reshape(ND, 128, 2, HC).transpose(1, 0, 2, 3)
        cm8 = np.ascontiguousarray(cv).reshape(128, ND * 2 * HC)
        cm8 = cm8.astype(NP_FP8)
        in_maps.append({"x8": x_all, "w8": w8, "cm8": cm8})
    return in_maps


def _run(samples, bhv_matrix, centroids, **spmd_kwargs):
    nc = _build_nc()
    in_maps = _prep_in_maps(samples, bhv_matrix, centroids)
    res = run_bass_kernel_spmd(nc, in_maps, core_ids=list(range(NCORES)),
                               **spmd_kwargs)
    acc = np.zeros((C, B), dtype=np.float32)
    for r in res.results:
        acc += r["out"]
    return np.ascontiguousarray(acc.T), res


def kernel(samples, bhv_matrix, centroids):
    out, _ = _run(samples, bhv_matrix, centroids)
    return out
